# Initial kernel scaffold
#
"""Trainium2 Bass kernel for the sparse-conv network (nn_ExampleNet).

Pipeline (per batch image): scatter 200k sparse voxel features into a dense
[256,256,32] grid, SparseConv(32->64) + 2x SubMConv(64) with an active-site
mask, SparseConvTranspose(64, stride 2), dense 3x3 VALID conv -> [511,511,64].

Strategy: 8-way SPMD (4 batches x 2 row-halves), row-parity-packed layout:
every on-chip tensor stores row pairs across the 128 partitions
(partitions 0:64 = even row channels, 64:128 = odd row), so every matmul
uses all 128 PE output lanes and every activation/mask-multiply runs at
128-partition throughput. Everything stored in bf16 (halves SBUF + DMA +
doubles DVE rate); PSUM accumulation stays fp32.
"""
from contextlib import ExitStack

import numpy as np
import ml_dtypes

import concourse.bacc as bacc
import concourse.mybir as mybir
import concourse.tile as tile
from concourse.bass_utils import run_bass_kernel_spmd

F32 = mybir.dt.float32
F32R = mybir.dt.float32r
BF16 = mybir.dt.bfloat16
RELU = mybir.ActivationFunctionType.Relu
BF = ml_dtypes.bfloat16

B, H, W = 4, 256, 256
NCOL = 258      # padded col count for x/h1-3 slabs
P4 = 516        # h4/m4 col pitch
CH = 64         # output rows per chunk
NCH = 256 // CH
SADV = CH // 4  # x/h slot advance per chunk
NUP = CH // 2 + 1           # h4 up-pairs per chunk
S3, S2, S1, SX = CH // 4 + 2, CH // 4 + 4, CH // 4 + 6, CH // 4 + 8

DELTAS9 = [(dlt, d) for dlt in (-1, 0, 1) for d in range(3)]
SIGD = [(sg, d) for sg in (0, 1) for d in range(3)]
# convT up-pair batches: (first local up-pair, count); even-l use the
# odd-aligned h3 copy, odd-l the standard one
TBATCH = ([(l0, min(2, (NUP - l0 + 1) // 2)) for l0 in range(0, NUP, 4)]
          + [(l0, min(2, (NUP - l0 + 1) // 2)) for l0 in range(1, NUP, 4)])

_CACHE = {}


def _pack_weights(w1, w2, w3, wt, w5):
    f32 = np.float32
    # conv1: m over x-row offset; p = ch + 32*cs (cs = dx); lane = ch'+64*op
    w1c = np.zeros((96, 4, 128), f32)
    for m in range(4):
        for cs in range(3):
            for op in range(2):
                dy = m - op
                if 0 <= dy < 3:
                    w1c[32 * cs:32 * cs + 32, m,
                        64 * op:64 * op + 64] = w1[dy, cs]

    def pack9(wm):
        out = np.zeros((128, 9, 128), f32)
        for k, (dlt, d) in enumerate(DELTAS9):
            for rp in range(2):
                for op in range(2):
                    dy = 2 * dlt + rp - op + 1
                    if 0 <= dy < 3:
                        out[64 * rp:64 * rp + 64, k,
                            64 * op:64 * op + 64] = wm[dy, d]
        return out

    wte = wt[::-1, ::-1]  # jax conv_transpose applies the flipped kernel
    wtc = np.zeros((128, 3, 128), f32)
    for k, dx in enumerate((0, 2, 1)):
        wtc[0:64, k, 0:64] = wte[2, dx]
        wtc[64:128, k, 0:64] = wte[0, dx]
        wtc[64:128, k, 64:128] = wte[1, dx]
    w5c = np.zeros((128, 6, 128), f32)
    for k, (sg, d) in enumerate(SIGD):
        for rp in range(2):
            for op in range(2):
                dy = 2 * sg + rp - op
                if 0 <= dy < 3:
                    w5c[64 * rp:64 * rp + 64, k,
                        64 * op:64 * op + 64] = w5[dy, d]
    return tuple(w.astype(BF) for w in
                 (w1c, pack9(w2), pack9(w3), wtc, w5c))


def _host_prep(features, coors, w1, b1, w2, b2, w3, b3, wt, bt, w5, b5):
    f32 = np.float32
    bi, yi, xi = coors[:, 0], coors[:, 1], coors[:, 2]
    flat = (bi.astype(np.int64) * H + yi) * W + xi
    dense = np.zeros((B * H * W, 32), f32)
    for c in range(32):
        dense[:, c] = np.bincount(flat, weights=features[:, c],
                                  minlength=B * H * W)
    dense = dense.reshape(B, H, W, 32)
    occ = np.bincount(flat, minlength=B * H * W).reshape(B, H, W) > 0
    m0p = np.zeros((B, H + 2, W + 2), bool)
    m0p[:, 1:-1, 1:-1] = occ
    m1 = np.zeros((B, H, W), bool)
    for dy in range(3):
        for dx in range(3):
            m1 |= m0p[:, dy:dy + H, dx:dx + W]
    m4 = np.zeros((B, 2 * H + 1, 2 * W + 1), bool)
    for dy in range(3):
        for dx in range(3):
            m4[:, dy:dy + 2 * H - 1:2, dx:dx + 2 * W - 1:2] |= m1

    w1c, w2c, w3c, wtc, w5c = _pack_weights(w1, w2, w3, wt, w5)
    biases = np.zeros((128, 5), f32)
    for i, bb in enumerate((b1, b2, b3, bt, b5)):
        biases[0:64, i] = bb
        biases[64:128, i] = bb

    in_maps = []
    for core in range(8):
        b, half = core // 2, core % 2
        # padded dense image: rows -8..275, cols -1..258
        Xp = np.zeros((284, 260, 32), f32)
        Xp[8:8 + H, 1:1 + W] = dense[b]
        xs = np.empty((96, 152, NCOL), BF)
        for cs in range(3):
            v = Xp[128 * half + 1:128 * half + 153, cs:cs + NCOL, :]
            xs[32 * cs:32 * cs + 32] = v.transpose(2, 0, 1).astype(BF)
        M1p = np.zeros((272, NCOL), f32)
        M1p[8:8 + H, 1:1 + W] = m1[b]
        m1d = np.empty((128, 70, NCOL), BF)
        for rp in range(2):
            v = M1p[128 * half + 2 + rp:128 * half + 2 + rp + 140:2, :]
            m1d[64 * rp:64 * rp + 64] = np.broadcast_to(
                v.astype(BF)[None], (64, 70, NCOL))
        M4p = np.zeros((520, P4), f32)
        M4p[0:513, 0:513] = m4[b]
        m4d = np.empty((128, 129, P4), BF)
        for rp in range(2):
            v = M4p[256 * half + rp:256 * half + rp + 258:2, :]
            m4d[64 * rp:64 * rp + 64] = np.broadcast_to(
                v.astype(BF)[None], (64, 129, P4))
        in_maps.append(dict(
            xs=np.ascontiguousarray(xs),
            m1d=np.ascontiguousarray(m1d),
            m4d=np.ascontiguousarray(m4d),
            w1c=w1c, w2c=w2c, w3c=w3c, wtc=wtc, w5c=w5c, biases=biases,
        ))
    return in_maps


def _build_program():
    nc = bacc.Bacc("TRN2", target_bir_lowering=False, debug=False,
                   enable_asserts=True, num_devices=8)

    xs_d = nc.dram_tensor("xs", [96, 152, NCOL], BF16,
                          kind="ExternalInput").ap()
    m1_d = nc.dram_tensor("m1d", [128, 70, NCOL], BF16,
                          kind="ExternalInput").ap()
    m4_d = nc.dram_tensor("m4d", [128, 129, P4], BF16,
                          kind="ExternalInput").ap()
    w1_d = nc.dram_tensor("w1c", [96, 4, 128], BF16,
                          kind="ExternalInput").ap()
    w2_d = nc.dram_tensor("w2c", [128, 9, 128], BF16,
                          kind="ExternalInput").ap()
    w3_d = nc.dram_tensor("w3c", [128, 9, 128], BF16,
                          kind="ExternalInput").ap()
    wt_d = nc.dram_tensor("wtc", [128, 3, 128], BF16,
                          kind="ExternalInput").ap()
    w5_d = nc.dram_tensor("w5c", [128, 6, 128], BF16,
                          kind="ExternalInput").ap()
    bias_d = nc.dram_tensor("biases", [128, 5], F32,
                            kind="ExternalInput").ap()
    out_d = nc.dram_tensor("out", [128, 128, 511], BF16,
                           kind="ExternalOutput").ap()

    with tile.TileContext(nc) as tc, ExitStack() as ctx:
        wp = ctx.enter_context(tc.tile_pool(name="wp", bufs=1))
        xp = ctx.enter_context(tc.tile_pool(name="xp", bufs=2))
        mp = ctx.enter_context(tc.tile_pool(name="mp", bufs=2))
        hp = ctx.enter_context(tc.tile_pool(name="hp", bufs=1))
        pp = ctx.enter_context(tc.tile_pool(name="pp", bufs=4, space="PSUM"))
        op = ctx.enter_context(tc.tile_pool(name="op", bufs=6))

        h1 = wp.tile([128, S1, NCOL], BF16, name="h1buf")
        h2 = wp.tile([128, S2, NCOL], BF16, name="h2buf")
        w1t = wp.tile([96, 4, 128], BF16, name="w1t")
        w2t = wp.tile([128, 9, 128], BF16, name="w2t")
        w3t = wp.tile([128, 9, 128], BF16, name="w3t")
        wtt = wp.tile([128, 3, 128], BF16, name="wtt")
        w5t = wp.tile([128, 6, 128], BF16, name="w5t")
        bt = wp.tile([128, 5], F32, name="bt")
        nc.sync.dma_start(w1t[:], w1_d[:])
        nc.sync.dma_start(bt[:], bias_d[:])
        nc.scalar.dma_start(w2t[:], w2_d[:])
        nc.scalar.dma_start(w3t[:], w3_d[:])
        nc.scalar.dma_start(wtt[:], wt_d[:])
        nc.scalar.dma_start(w5t[:], w5_d[:])

        def conv1_layer(x2, s_lo, m_ch, h_out):
            nc.gpsimd.memset(h_out[:, :, 0:NCOL:NCOL - 1], 0)
            for t0 in range(s_lo, S1, 2):
                nt = min(2, S1 - t0)
                pc = pp.tile([128, 2, 256], F32, name="pc", tag="ps")
                ps = pc[:, 0:nt, :]
                for m in range(4):
                    nc.tensor.matmul(
                        ps, w1t[:, m, :],
                        x2[:, 2 * t0 + m:2 * t0 + m + 2 * nt - 1:2, 0:256],
                        start=(m == 0), stop=(m == 3))
                dst = h_out[:, t0:t0 + nt, 1:257]
                nc.scalar.activation(dst, ps, RELU, bias=bt[:, 0:1])
                nc.vector.tensor_mul(dst, dst, m_ch[:, t0:t0 + nt, 1:257])

        def conv_layer(inp, wt_, deltas, s_lo, nslots, bias_ap, m_ch, moff,
                       h_out):
            nc.gpsimd.memset(h_out[:, :, 0:NCOL:NCOL - 1], 0)
            for t0 in range(s_lo, nslots, 2):
                nt = min(2, nslots - t0)
                pc = pp.tile([128, 2, 256], F32, name="pc", tag="ps")
                ps = pc[:, 0:nt, :]
                for k, (dlt, d) in enumerate(deltas):
                    nc.tensor.matmul(
                        ps, wt_[:, k, :],
                        inp[:, t0 + 1 + dlt:t0 + 1 + dlt + nt, d:d + 256],
                        start=(k == 0), stop=(k == len(deltas) - 1))
                dst = h_out[:, t0:t0 + nt, 1:257]
                nc.scalar.activation(dst, ps, RELU, bias=bias_ap)
                nc.vector.tensor_mul(
                    dst, dst, m_ch[:, t0 + moff:t0 + moff + nt, 1:257])

        NXR = 2 * S1 + 2   # x rows per chunk
        def load_chunk(c):
            x_ch = xp.tile([96, NXR, NCOL], BF16, name="x_ch", tag="x",
                           bufs=1)
            r0 = 2 * SADV * c
            nc.sync.dma_start(x_ch[:, 0:8, :], xs_d[:, r0:r0 + 8, :])
            nc.sync.dma_start(x_ch[:, 8:24, :], xs_d[:, r0 + 8:r0 + 24, :])
            nc.sync.dma_start(x_ch[:, 24:NXR, :], xs_d[:, r0 + 24:r0 + NXR, :])
            m1_ch = mp.tile([128, S1, NCOL], BF16, name="m1_ch", tag="m1")
            nc.gpsimd.dma_start(m1_ch[:, 0:8, :],
                                m1_d[:, SADV * c:SADV * c + 8, :])
            nc.gpsimd.dma_start(m1_ch[:, 8:S1, :],
                                m1_d[:, SADV * c + 8:SADV * c + S1, :])
            return x_ch, m1_ch

        def load_m4(c):
            # deliberately emitted after conv1 so its bulk transfer queues
            # behind conv1's activations and never starves the startup loads
            m4_ch = mp.tile([128, NUP, P4], BF16, name="m4_ch", tag="m4",
                            bufs=1)
            nc.scalar.dma_start(m4_ch[:], m4_d[:, (CH // 2) * c:
                                               (CH // 2) * c + NUP, :])
            return m4_ch

        def emit_convT(h3, h3o, h4, m4_ch):
            for l0, nb in TBATCH:
                T, ii = (h3o, l0 // 2) if l0 % 2 == 0 else (h3, (l0 + 1) // 2)
                pe = pp.tile([128, 2, 512], F32, name="pe", tag="ps")
                for q in range(nb):
                    nc.tensor.matmul(pe[:, q, 0:257], wtt[:, 0, :],
                                     T[:, ii + q, 1:258],
                                     start=True, stop=False)
                    nc.tensor.matmul(pe[:, q, 0:257], wtt[:, 1, :],
                                     T[:, ii + q, 0:257],
                                     start=False, stop=True)
                de = h4[:, l0:l0 + 2 * nb - 1:2, 0:513:2]
                nc.scalar.activation(de, pe[:, 0:nb, 0:257], RELU,
                                     bias=bt[:, 3:4])
                po = pp.tile([128, 2, 256], F32, name="po", tag="ps")
                nc.tensor.matmul(po[:, 0:nb, :], wtt[:, 2, :],
                                 T[:, ii:ii + nb, 1:257],
                                 start=True, stop=True)
                do = h4[:, l0:l0 + 2 * nb - 1:2, 1:512:2]
                nc.scalar.activation(do, po[:, 0:nb, :], RELU, bias=bt[:, 3:4])
            for p0 in range(0, NUP, (NUP + 3) // 4):
                p1 = min(NUP, p0 + (NUP + 3) // 4)
                nc.vector.tensor_mul(h4[:, p0:p1, 0:513], h4[:, p0:p1, 0:513],
                                     m4_ch[:, p0:p1, 0:513])

        def emit_conv5(h4, c, r0s):
            for r0 in r0s:
                out_sb = op.tile([128, 2, 511], BF16, name="out_sb", tag="o")
                p5 = pp.tile([128, 2, 512], F32, name="p5", tag="ps")
                for q in range(2):
                    for k, (sg, d) in enumerate(SIGD):
                        nc.tensor.matmul(p5[:, q, :], w5t[:, k, :],
                                         h4[:, r0 + q + sg, d:d + 512],
                                         start=(k == 0), stop=(k == 5))
                nc.scalar.activation(out_sb[:], p5[:, :, 0:511], RELU,
                                     bias=bt[:, 4:5])
                eng = nc.gpsimd if (r0 // 2) % 2 == 0 else nc.sync
                eng.dma_start(
                    out_d[:, (CH // 2) * c + r0:(CH // 2) * c + r0 + 2, :],
                    out_sb[:])

        prev = None  # (h3, h3o, h4, m4_ch, c)
        h3prev = None
        nxt = load_chunk(0)
        for it in range(NCH + 1):
            if it < NCH:
                x_ch, m1_ch = nxt
                if it + 1 < NCH:
                    nxt = load_chunk(it + 1)
                h3 = hp.tile([128, S3, NCOL], BF16, name="h3", tag="h3",
                             bufs=2)
                h3o = hp.tile([128, S3 - 1, NCOL], BF16, name="h3o",
                              tag="h3o", bufs=2)
                # carry the exact boundary slots from the previous chunk
                # instead of recomputing them (h1/h2 are persistent tiles;
                # in-place copies between disjoint slot ranges)
                ov1, ov2, ov3 = S1 - SADV, S2 - SADV, S3 - SADV
                if it > 0:
                    nc.vector.tensor_copy(h1[:, 0:ov1, :],
                                          h1[:, SADV:S1, :])
                    nc.vector.tensor_copy(h2[:, 0:ov2, :],
                                          h2[:, SADV:S2, :])
                    nc.vector.tensor_copy(h3[:, 0:ov3, :],
                                          h3prev[:, SADV:S3, :])
                s1, s2, s3 = (ov1, ov2, ov3) if it > 0 else (0, 0, 0)
                conv1_layer(x_ch, s1, m1_ch, h1)
                m4_ch = load_m4(it)
            if prev is not None:
                h3p, h3op, h4p, m4p_, cp = prev
                emit_convT(h3p, h3op, h4p, m4p_)
            if it < NCH:
                conv_layer(h1, w2t, DELTAS9, s2, S2, bt[:, 1:2], m1_ch, 1, h2)
            if prev is not None:
                emit_conv5(h4p, cp, range(0, CH // 4, 2))
            if it < NCH:
                conv_layer(h2, w3t, DELTAS9, s3, S3, bt[:, 2:3], m1_ch, 2, h3)
                nc.vector.tensor_copy(h3o[0:64, :, :],
                                      h3[64:128, 0:S3 - 1, :])
                nc.vector.tensor_copy(h3o[64:128, :, :], h3[0:64, 1:S3, :])
            if prev is not None:
                emit_conv5(h4p, cp, range(CH // 4, CH // 2, 2))
            if it < NCH:
                h4 = hp.tile([128, NUP, P4], BF16, name="h4", tag="h4")
                prev = (h3, h3o, h4, m4_ch, it)
                h3prev = h3
            else:
                prev = None

    nc.compile()
    return nc


def kernel(**inputs):
    features = np.asarray(inputs["features"], np.float32)
    coors = np.asarray(inputs["coors"], np.int32)
    args = [np.asarray(inputs[k], np.float32) for k in
            ("w1", "b1", "w2", "b2", "w3", "b3", "wt", "bt", "w5", "b5")]
    in_maps = _host_prep(features, coors, *args)
    if "nc" not in _CACHE:
        _CACHE["nc"] = _build_program()
    res = run_bass_kernel_spmd(_CACHE["nc"], in_maps,
                               core_ids=list(range(8)), trace=False)
    full = np.zeros((B, 511, 511, 64), np.float32)
    for core in range(8):
        b, half = core // 2, core % 2
        o = np.asarray(res.results[core]["out"]).astype(np.float32)
        rows = o.reshape(2, 64, 128, 511).transpose(2, 0, 3, 1) \
            .reshape(256, 511, 64)
        nrow = 256 if half == 0 else 255
        full[b, 256 * half:256 * half + nrow] = rows[:nrow]
    return full



# revision 1
# speedup vs baseline: 1.3763x; 1.3763x over previous
"""Trainium2 Bass kernel for the sparse-conv network (nn_ExampleNet).

Pipeline (per batch image): scatter 200k sparse voxel features into a dense
[256,256,32] grid, SparseConv(32->64) + 2x SubMConv(64) with an active-site
mask, SparseConvTranspose(64, stride 2), dense 3x3 VALID conv -> [511,511,64].

Strategy: 8-way SPMD (4 batches x 2 row-halves), row-parity-packed layout:
every on-chip tensor stores row pairs across the 128 partitions
(partitions 0:64 = even row channels, 64:128 = odd row), so every matmul
uses all 128 PE output lanes and every activation/mask-multiply runs at
128-partition throughput. Everything stored in bf16 (halves SBUF + DMA +
doubles DVE rate); PSUM accumulation stays fp32.
"""
from contextlib import ExitStack

import numpy as np
import ml_dtypes

import concourse.bacc as bacc
import concourse.mybir as mybir
import concourse.tile as tile
from concourse.bass_utils import run_bass_kernel_spmd

F32 = mybir.dt.float32
F32R = mybir.dt.float32r
BF16 = mybir.dt.bfloat16
RELU = mybir.ActivationFunctionType.Relu
BF = ml_dtypes.bfloat16

B, H, W = 4, 256, 256
NCOL = 258      # padded col count for x/h1-3 slabs
P4 = 516        # h4/m4 col pitch
CH = 64         # output rows per chunk
NCH = 256 // CH
SADV = CH // 4  # x/h slot advance per chunk
NUP = CH // 2 + 1           # h4 up-pairs per chunk
S3, S2, S1, SX = CH // 4 + 2, CH // 4 + 4, CH // 4 + 6, CH // 4 + 8

DELTAS9 = [(dlt, d) for dlt in (-1, 0, 1) for d in range(3)]
SIGD = [(sg, d) for sg in (0, 1) for d in range(3)]
# convT up-pair batches: (first local up-pair, count); even-l use the
# odd-aligned h3 copy, odd-l the standard one
TBATCH = ([(l0, min(2, (NUP - l0 + 1) // 2)) for l0 in range(0, NUP, 4)]
          + [(l0, min(2, (NUP - l0 + 1) // 2)) for l0 in range(1, NUP, 4)])

_CACHE = {}


def _pack_weights(w1, w2, w3, wt, w5):
    f32 = np.float32
    # conv1: m over x-row offset; p = ch + 32*cs (cs = dx); lane = ch'+64*op
    w1c = np.zeros((96, 4, 128), f32)
    for m in range(4):
        for cs in range(3):
            for op in range(2):
                dy = m - op
                if 0 <= dy < 3:
                    w1c[32 * cs:32 * cs + 32, m,
                        64 * op:64 * op + 64] = w1[dy, cs]

    def pack9(wm):
        out = np.zeros((128, 9, 128), f32)
        for k, (dlt, d) in enumerate(DELTAS9):
            for rp in range(2):
                for op in range(2):
                    dy = 2 * dlt + rp - op + 1
                    if 0 <= dy < 3:
                        out[64 * rp:64 * rp + 64, k,
                            64 * op:64 * op + 64] = wm[dy, d]
        return out

    wte = wt[::-1, ::-1]  # jax conv_transpose applies the flipped kernel
    wtc = np.zeros((128, 3, 128), f32)
    for k, dx in enumerate((0, 2, 1)):
        wtc[0:64, k, 0:64] = wte[2, dx]
        wtc[64:128, k, 0:64] = wte[0, dx]
        wtc[64:128, k, 64:128] = wte[1, dx]
    w5c = np.zeros((128, 6, 128), f32)
    for k, (sg, d) in enumerate(SIGD):
        for rp in range(2):
            for op in range(2):
                dy = 2 * sg + rp - op
                if 0 <= dy < 3:
                    w5c[64 * rp:64 * rp + 64, k,
                        64 * op:64 * op + 64] = w5[dy, d]
    return tuple(w.astype(BF) for w in
                 (w1c, pack9(w2), pack9(w3), wtc, w5c))


def _host_prep(features, coors, w1, b1, w2, b2, w3, b3, wt, bt, w5, b5):
    f32 = np.float32
    bi, yi, xi = coors[:, 0], coors[:, 1], coors[:, 2]
    flat = (bi.astype(np.int64) * H + yi) * W + xi
    dense = np.zeros((B * H * W, 32), f32)
    for c in range(32):
        dense[:, c] = np.bincount(flat, weights=features[:, c],
                                  minlength=B * H * W)
    dense = dense.reshape(B, H, W, 32)
    occ = np.bincount(flat, minlength=B * H * W).reshape(B, H, W) > 0
    m0p = np.zeros((B, H + 2, W + 2), bool)
    m0p[:, 1:-1, 1:-1] = occ
    m1 = np.zeros((B, H, W), bool)
    for dy in range(3):
        for dx in range(3):
            m1 |= m0p[:, dy:dy + H, dx:dx + W]
    m4 = np.zeros((B, 2 * H + 1, 2 * W + 1), bool)
    for dy in range(3):
        for dx in range(3):
            m4[:, dy:dy + 2 * H - 1:2, dx:dx + 2 * W - 1:2] |= m1

    w1c, w2c, w3c, wtc, w5c = _pack_weights(w1, w2, w3, wt, w5)
    biases = np.zeros((128, 5), f32)
    for i, bb in enumerate((b1, b2, b3, bt, b5)):
        biases[0:64, i] = bb
        biases[64:128, i] = bb

    in_maps = []
    for core in range(8):
        b, half = core // 2, core % 2
        # padded dense image: rows -8..275, cols -1..258
        Xp = np.zeros((284, 260, 32), f32)
        Xp[8:8 + H, 1:1 + W] = dense[b]
        xs = np.empty((96, 152, NCOL), BF)
        for cs in range(3):
            v = Xp[128 * half + 1:128 * half + 153, cs:cs + NCOL, :]
            xs[32 * cs:32 * cs + 32] = v.transpose(2, 0, 1).astype(BF)
        M1p = np.zeros((272, NCOL), f32)
        M1p[8:8 + H, 1:1 + W] = m1[b]
        m1d = np.empty((128, 70, NCOL), BF)
        for rp in range(2):
            v = M1p[128 * half + 2 + rp:128 * half + 2 + rp + 140:2, :]
            m1d[64 * rp:64 * rp + 64] = np.broadcast_to(
                v.astype(BF)[None], (64, 70, NCOL))
        M4p = np.zeros((520, P4), f32)
        M4p[0:513, 0:513] = m4[b]
        m4d = np.empty((128, 129, P4), BF)
        for rp in range(2):
            v = M4p[256 * half + rp:256 * half + rp + 258:2, :]
            m4d[64 * rp:64 * rp + 64] = np.broadcast_to(
                v.astype(BF)[None], (64, 129, P4))
        in_maps.append(dict(
            xs=np.ascontiguousarray(xs),
            m1d=np.ascontiguousarray(m1d),
            m4d=np.ascontiguousarray(m4d),
            w1c=w1c, w2c=w2c, w3c=w3c, wtc=wtc, w5c=w5c, biases=biases,
        ))
    return in_maps


def _build_program():
    nc = bacc.Bacc("TRN2", target_bir_lowering=False, debug=False,
                   enable_asserts=True, num_devices=8)

    xs_d = nc.dram_tensor("xs", [96, 152, NCOL], BF16,
                          kind="ExternalInput").ap()
    m1_d = nc.dram_tensor("m1d", [128, 70, NCOL], BF16,
                          kind="ExternalInput").ap()
    m4_d = nc.dram_tensor("m4d", [128, 129, P4], BF16,
                          kind="ExternalInput").ap()
    w1_d = nc.dram_tensor("w1c", [96, 4, 128], BF16,
                          kind="ExternalInput").ap()
    w2_d = nc.dram_tensor("w2c", [128, 9, 128], BF16,
                          kind="ExternalInput").ap()
    w3_d = nc.dram_tensor("w3c", [128, 9, 128], BF16,
                          kind="ExternalInput").ap()
    wt_d = nc.dram_tensor("wtc", [128, 3, 128], BF16,
                          kind="ExternalInput").ap()
    w5_d = nc.dram_tensor("w5c", [128, 6, 128], BF16,
                          kind="ExternalInput").ap()
    bias_d = nc.dram_tensor("biases", [128, 5], F32,
                            kind="ExternalInput").ap()
    out_d = nc.dram_tensor("out", [128, 128, 511], BF16,
                           kind="ExternalOutput").ap()

    with tile.TileContext(nc) as tc, ExitStack() as ctx:
        wp = ctx.enter_context(tc.tile_pool(name="wp", bufs=1))
        xp = ctx.enter_context(tc.tile_pool(name="xp", bufs=2))
        mp = ctx.enter_context(tc.tile_pool(name="mp", bufs=2))
        hp = ctx.enter_context(tc.tile_pool(name="hp", bufs=1))
        pp = ctx.enter_context(tc.tile_pool(name="pp", bufs=4, space="PSUM"))
        op = ctx.enter_context(tc.tile_pool(name="op", bufs=6))

        h1 = wp.tile([128, S1, NCOL], BF16, name="h1buf")
        h2 = wp.tile([128, S2, NCOL], BF16, name="h2buf")
        w1t = wp.tile([96, 4, 128], BF16, name="w1t")
        w2t = wp.tile([128, 9, 128], BF16, name="w2t")
        w3t = wp.tile([128, 9, 128], BF16, name="w3t")
        wtt = wp.tile([128, 3, 128], BF16, name="wtt")
        w5t = wp.tile([128, 6, 128], BF16, name="w5t")
        bt = wp.tile([128, 5], F32, name="bt")
        nc.sync.dma_start(w1t[:], w1_d[:])
        nc.sync.dma_start(bt[:], bias_d[:])
        nc.scalar.dma_start(w2t[:], w2_d[:])
        nc.scalar.dma_start(w3t[:], w3_d[:])
        nc.scalar.dma_start(wtt[:], wt_d[:])
        nc.scalar.dma_start(w5t[:], w5_d[:])

        def conv1_layer(x2, s_lo, m_ch, h_out):
            nc.gpsimd.memset(h_out[:, :, 0:NCOL:NCOL - 1], 0)
            for t0 in range(s_lo, S1, 2):
                nt = min(2, S1 - t0)
                pc = pp.tile([128, 2, 256], F32, name="pc", tag="ps")
                ps = pc[:, 0:nt, :]
                for m in range(4):
                    nc.tensor.matmul(
                        ps, w1t[:, m, :],
                        x2[:, 2 * t0 + m:2 * t0 + m + 2 * nt - 1:2, 0:256],
                        start=(m == 0), stop=(m == 3))
                dst = h_out[:, t0:t0 + nt, 1:257]
                nc.scalar.activation(dst, ps, RELU, bias=bt[:, 0:1])
                nc.vector.tensor_mul(dst, dst, m_ch[:, t0:t0 + nt, 1:257])

        def conv_layer(inp, wt_, deltas, s_lo, nslots, bias_ap, m_ch, moff,
                       h_out):
            nc.gpsimd.memset(h_out[:, :, 0:NCOL:NCOL - 1], 0)
            for t0 in range(s_lo, nslots, 2):
                nt = min(2, nslots - t0)
                pc = pp.tile([128, 2, 256], F32, name="pc", tag="ps")
                ps = pc[:, 0:nt, :]
                for k, (dlt, d) in enumerate(deltas):
                    nc.tensor.matmul(
                        ps, wt_[:, k, :],
                        inp[:, t0 + 1 + dlt:t0 + 1 + dlt + nt, d:d + 256],
                        start=(k == 0), stop=(k == len(deltas) - 1))
                dst = h_out[:, t0:t0 + nt, 1:257]
                nc.scalar.activation(dst, ps, RELU, bias=bias_ap)
                nc.vector.tensor_mul(
                    dst, dst, m_ch[:, t0 + moff:t0 + moff + nt, 1:257])

        NXR = 2 * S1 + 2   # x rows per chunk
        def load_chunk(c):
            x_ch = xp.tile([96, NXR, NCOL], BF16, name="x_ch", tag="x",
                           bufs=1)
            r0 = 2 * SADV * c
            nc.sync.dma_start(x_ch[:, 0:8, :], xs_d[:, r0:r0 + 8, :])
            nc.sync.dma_start(x_ch[:, 8:24, :], xs_d[:, r0 + 8:r0 + 24, :])
            nc.sync.dma_start(x_ch[:, 24:NXR, :], xs_d[:, r0 + 24:r0 + NXR, :])
            m1_ch = mp.tile([128, S1, NCOL], BF16, name="m1_ch", tag="m1")
            nc.gpsimd.dma_start(m1_ch[:, 0:8, :],
                                m1_d[:, SADV * c:SADV * c + 8, :])
            nc.gpsimd.dma_start(m1_ch[:, 8:S1, :],
                                m1_d[:, SADV * c + 8:SADV * c + S1, :])
            return x_ch, m1_ch

        def load_m4(c):
            # deliberately emitted after conv1 so its bulk transfer queues
            # behind conv1's activations and never starves the startup loads
            m4_ch = mp.tile([128, NUP, P4], BF16, name="m4_ch", tag="m4",
                            bufs=1)
            nc.scalar.dma_start(m4_ch[:], m4_d[:, (CH // 2) * c:
                                               (CH // 2) * c + NUP, :])
            return m4_ch

        def emit_convT(h3, h3o, h4, m4_ch):
            for l0, nb in TBATCH:
                T, ii = (h3o, l0 // 2) if l0 % 2 == 0 else (h3, (l0 + 1) // 2)
                pe = pp.tile([128, 2, 512], F32, name="pe", tag="ps")
                for q in range(nb):
                    nc.tensor.matmul(pe[:, q, 0:257], wtt[:, 0, :],
                                     T[:, ii + q, 1:258],
                                     start=True, stop=False)
                    nc.tensor.matmul(pe[:, q, 0:257], wtt[:, 1, :],
                                     T[:, ii + q, 0:257],
                                     start=False, stop=True)
                de = h4[:, l0:l0 + 2 * nb - 1:2, 0:513:2]
                nc.scalar.activation(de, pe[:, 0:nb, 0:257], RELU,
                                     bias=bt[:, 3:4])
                po = pp.tile([128, 2, 256], F32, name="po", tag="ps")
                nc.tensor.matmul(po[:, 0:nb, :], wtt[:, 2, :],
                                 T[:, ii:ii + nb, 1:257],
                                 start=True, stop=True)
                do = h4[:, l0:l0 + 2 * nb - 1:2, 1:512:2]
                nc.scalar.activation(do, po[:, 0:nb, :], RELU, bias=bt[:, 3:4])
            for p0 in range(0, NUP, (NUP + 3) // 4):
                p1 = min(NUP, p0 + (NUP + 3) // 4)
                nc.vector.tensor_mul(h4[:, p0:p1, 0:513], h4[:, p0:p1, 0:513],
                                     m4_ch[:, p0:p1, 0:513])

        def emit_conv5(h4, c, r0s):
            for r0 in r0s:
                out_sb = op.tile([128, 2, 511], BF16, name="out_sb", tag="o")
                p5 = pp.tile([128, 2, 512], F32, name="p5", tag="ps")
                for q in range(2):
                    for k, (sg, d) in enumerate(SIGD):
                        nc.tensor.matmul(p5[:, q, :], w5t[:, k, :],
                                         h4[:, r0 + q + sg, d:d + 512],
                                         start=(k == 0), stop=(k == 5))
                nc.scalar.activation(out_sb[:], p5[:, :, 0:511], RELU,
                                     bias=bt[:, 4:5])
                eng = nc.gpsimd if (r0 // 2) % 2 == 0 else nc.sync
                eng.dma_start(
                    out_d[:, (CH // 2) * c + r0:(CH // 2) * c + r0 + 2, :],
                    out_sb[:])

        prev = None  # (h3, h3o, h4, m4_ch, c)
        h3prev = None
        nxt = load_chunk(0)
        for it in range(NCH + 1):
            if it < NCH:
                x_ch, m1_ch = nxt
                if it + 1 < NCH:
                    nxt = load_chunk(it + 1)
                h3 = hp.tile([128, S3, NCOL], BF16, name="h3", tag="h3",
                             bufs=2)
                h3o = hp.tile([128, S3 - 1, NCOL], BF16, name="h3o",
                              tag="h3o", bufs=2)
                # carry the exact boundary slots from the previous chunk
                # instead of recomputing them (h1/h2 are persistent tiles;
                # in-place copies between disjoint slot ranges)
                ov1, ov2, ov3 = S1 - SADV, S2 - SADV, S3 - SADV
                if it > 0:
                    nc.vector.tensor_copy(h1[:, 0:ov1, :],
                                          h1[:, SADV:S1, :])
                    nc.vector.tensor_copy(h2[:, 0:ov2, :],
                                          h2[:, SADV:S2, :])
                    nc.vector.tensor_copy(h3[:, 0:ov3, :],
                                          h3prev[:, SADV:S3, :])
                s1, s2, s3 = (ov1, ov2, ov3) if it > 0 else (0, 0, 0)
                conv1_layer(x_ch, s1, m1_ch, h1)
                m4_ch = load_m4(it)
            if prev is not None:
                h3p, h3op, h4p, m4p_, cp = prev
                emit_convT(h3p, h3op, h4p, m4p_)
            if it < NCH:
                conv_layer(h1, w2t, DELTAS9, s2, S2, bt[:, 1:2], m1_ch, 1, h2)
            if prev is not None:
                emit_conv5(h4p, cp, range(0, CH // 4, 2))
            if it < NCH:
                conv_layer(h2, w3t, DELTAS9, s3, S3, bt[:, 2:3], m1_ch, 2, h3)
                nc.vector.tensor_copy(h3o[0:64, :, :],
                                      h3[64:128, 0:S3 - 1, :])
                nc.vector.tensor_copy(h3o[64:128, :, :], h3[0:64, 1:S3, :])
            if prev is not None:
                emit_conv5(h4p, cp, range(CH // 4, CH // 2, 2))
            if it < NCH:
                h4 = hp.tile([128, NUP, P4], BF16, name="h4", tag="h4")
                prev = (h3, h3o, h4, m4_ch, it)
                h3prev = h3
            else:
                prev = None

    nc.compile()
    return nc


def kernel(**inputs):
    features = np.asarray(inputs["features"], np.float32)
    coors = np.asarray(inputs["coors"], np.int32)
    args = [np.asarray(inputs[k], np.float32) for k in
            ("w1", "b1", "w2", "b2", "w3", "b3", "wt", "bt", "w5", "b5")]
    in_maps = _host_prep(features, coors, *args)
    if "nc" not in _CACHE:
        _CACHE["nc"] = _build_program()
    res = run_bass_kernel_spmd(_CACHE["nc"], in_maps,
                               core_ids=list(range(8)), trace=False)
    full = np.zeros((B, 511, 511, 64), np.float32)
    for core in range(8):
        b, half = core // 2, core % 2
        o = np.asarray(res.results[core]["out"]).astype(np.float32)
        rows = o.reshape(2, 64, 128, 511).transpose(2, 0, 3, 1) \
            .reshape(256, 511, 64)
        nrow = 256 if half == 0 else 255
        full[b, 256 * half:256 * half + nrow] = rows[:nrow]
    return full



# revision 9
# speedup vs baseline: 1.5502x; 1.1264x over previous
"""Trainium2 Bass kernel for the sparse-conv network (nn_ExampleNet).

Pipeline (per batch image): scatter 200k sparse voxel features into a dense
[256,256,32] grid, SparseConv(32->64) + 2x SubMConv(64) with an active-site
mask, SparseConvTranspose(64, stride 2), dense 3x3 VALID conv -> [511,511,64].

Strategy: 8-way SPMD (4 batches x 2 row-halves), row-parity-packed layout:
every on-chip tensor stores row pairs across the 128 partitions
(partitions 0:64 = even row channels, 64:128 = odd row), so every matmul
uses all 128 PE output lanes. Row taps are decomposed into 3 dense
k-steps (full 128x128 weights) + 3 staggered k-steps reading a
parity-staggered copy of the input (rp0 half advanced one slot, rp1 half
delayed one slot), cutting conv2/3 from 9 to 6 matmul steps. conv1 packs
[dense 32ch x 2 parity | staggered] into the 128 contraction lanes for 3
k-steps. Everything stored in bf16; PSUM accumulation stays fp32.
"""
from contextlib import ExitStack

import numpy as np
import ml_dtypes

import concourse.bacc as bacc
import concourse.mybir as mybir
import concourse.tile as tile
from concourse.ap import AP
from concourse.bass_utils import run_bass_kernel_spmd

F32 = mybir.dt.float32
BF16 = mybir.dt.bfloat16
RELU = mybir.ActivationFunctionType.Relu
ADD = mybir.AluOpType.add
MAX = mybir.AluOpType.max
BF = ml_dtypes.bfloat16

B, H, W = 4, 256, 256
NCOL = 258      # padded col count for x/h1-3 slabs
P4 = 516        # h4/m4 col pitch
CH = 64         # output rows per chunk
NCH = 256 // CH
SADV = CH // 4  # x/h slot advance per chunk
NUP = CH // 2 + 1           # h4 up-pairs per chunk
S3, S2, S1, SX = CH // 4 + 2, CH // 4 + 4, CH // 4 + 6, CH // 4 + 8
NSLOT = 3 * SADV + S1       # global out-slot count for conv1 (72)

import os
USE_STAG2 = os.environ.get("STAG2", "1") == "1"
USE_STAG3 = os.environ.get("STAG3", "1") == "1"
DELTAS9 = [(dlt, d) for dlt in (-1, 0, 1) for d in range(3)]
SIGD = [(sg, d) for sg in (0, 1) for d in range(3)]
# convT up-pair batches: (first local up-pair, count); even-l use the
# odd-aligned h3 copy, odd-l the standard one
TBATCH = ([(l0, min(2, (NUP - l0 + 1) // 2)) for l0 in range(0, NUP, 4)]
          + [(l0, min(2, (NUP - l0 + 1) // 2)) for l0 in range(1, NUP, 4)])

_CACHE = {}


def _pack_weights(w1, w2, w3, wt, w5):
    f32 = np.float32
    # conv1: contraction 128 = [32ch x {row Y+0, Y+1} | staggered {Y-1, Y+2}]
    # lanes = ch' + 64*op (op = out-row parity); 3 k-steps over dx=d
    w1d = np.zeros((128, 3, 128), f32)
    for d in range(3):
        # grp0 (p 0:32)  = x row Y+0: op0 <- w1[1], op1 <- w1[0]
        # grp1 (p 32:64) = x row Y+1: op0 <- w1[2], op1 <- w1[1]
        # grp2 (p 64:96) = x row Y-1: op0 <- w1[0]
        # grp3 (p 96:128)= x row Y+2: op1 <- w1[2]
        w1d[0:32, d, 0:64] = w1[1, d]
        w1d[0:32, d, 64:128] = w1[0, d]
        w1d[32:64, d, 0:64] = w1[2, d]
        w1d[32:64, d, 64:128] = w1[1, d]
        w1d[64:96, d, 0:64] = w1[0, d]
        w1d[96:128, d, 64:128] = w1[2, d]

    def pack6(wm):
        # dense steps: quadrant (rp -> op) = wm[1 + rp - op, d]
        # stagger steps: g rp0-half holds slot+1 -> op1 tap wm[2, d];
        #                g rp1-half holds slot-1 -> op0 tap wm[0, d]
        out = np.zeros((128, 6, 128), f32)
        for d in range(3):
            out[0:64, d, 0:64] = wm[1, d]
            out[64:128, d, 0:64] = wm[2, d]
            out[0:64, d, 64:128] = wm[0, d]
            out[64:128, d, 64:128] = wm[1, d]
            out[0:64, 3 + d, 64:128] = wm[2, d]
            out[64:128, 3 + d, 0:64] = wm[0, d]
        return out

    wte = wt[::-1, ::-1]  # jax conv_transpose applies the flipped kernel
    wtc = np.zeros((128, 3, 128), f32)
    for k, dx in enumerate((0, 2, 1)):
        wtc[0:64, k, 0:64] = wte[2, dx]
        wtc[64:128, k, 0:64] = wte[0, dx]
        wtc[64:128, k, 64:128] = wte[1, dx]
    w5c = np.zeros((128, 6, 128), f32)
    for k, (sg, d) in enumerate(SIGD):
        for rp in range(2):
            for op in range(2):
                dy = 2 * sg + rp - op
                if 0 <= dy < 3:
                    w5c[64 * rp:64 * rp + 64, k,
                        64 * op:64 * op + 64] = w5[dy, d]
    def pack9(wm):
        out = np.zeros((128, 9, 128), f32)
        for k, (dlt, d) in enumerate(DELTAS9):
            for rp in range(2):
                for op in range(2):
                    dy = 2 * dlt + rp - op + 1
                    if 0 <= dy < 3:
                        out[64 * rp:64 * rp + 64, k,
                            64 * op:64 * op + 64] = wm[dy, d]
        return out

    p2 = pack6(w2) if USE_STAG2 else pack9(w2)
    p3 = pack6(w3) if USE_STAG3 else pack9(w3)
    return tuple(w.astype(BF) for w in (w1d, p2, p3, wtc, w5c))


def _host_prep(features, coors, w1, b1, w2, b2, w3, b3, wt, bt, w5, b5):
    f32 = np.float32
    bi, yi, xi = coors[:, 0], coors[:, 1], coors[:, 2]
    flat = (bi.astype(np.int64) * H + yi) * W + xi
    dense = np.zeros((B * H * W, 32), f32)
    for c in range(32):
        dense[:, c] = np.bincount(flat, weights=features[:, c],
                                  minlength=B * H * W)
    dense = dense.reshape(B, H, W, 32)
    occ = np.bincount(flat, minlength=B * H * W).reshape(B, H, W) > 0
    m0p = np.zeros((B, H + 2, W + 2), bool)
    m0p[:, 1:-1, 1:-1] = occ
    m1 = np.zeros((B, H, W), bool)
    for dy in range(3):
        for dx in range(3):
            m1 |= m0p[:, dy:dy + H, dx:dx + W]
    m4 = np.zeros((B, 2 * H + 1, 2 * W + 1), bool)
    for dy in range(3):
        for dx in range(3):
            m4[:, dy:dy + 2 * H - 1:2, dx:dx + 2 * W - 1:2] |= m1

    w1d, w2c, w3c, wtc, w5c = _pack_weights(w1, w2, w3, wt, w5)
    biases = np.zeros((128, 5), f32)
    for i, bb in enumerate((b1, b2, b3, bt, b5)):
        biases[0:64, i] = bb
        biases[64:128, i] = bb

    in_maps = []
    for core in range(8):
        b, half = core // 2, core % 2
        # padded dense image: rows -8..275, cols -1..258
        Xp = np.zeros((284, 260, 32), f32)
        Xp[8:8 + H, 1:1 + W] = dense[b]
        # slab rows r (0..151) <-> image row y = 128*half + r - 7
        slab = Xp[128 * half + 1:128 * half + 153, 0:NCOL, :]  # [152,258,32]
        slab = slab.transpose(2, 0, 1).astype(BF)              # [32,152,258]
        # conv1 moving layout: out slot s covers rows Y=2s-6(+128h)+{0,1};
        # grp0..3 = x rows (Y+0, Y+1, Y-1, Y+2) = slab rows 2s+1,2s+2,2s,2s+3
        xd = np.zeros((128, NSLOT, NCOL), BF)
        smax = NSLOT  # 72 slots; slab row max = 2*71+3 = 145 < 152
        for g, dr in enumerate((1, 2, 0, 3)):
            v = slab[:, dr:dr + 2 * smax - 1:2, :]
            xd[32 * g:32 * g + 32, :v.shape[1]] = v
        M1p = np.zeros((272, NCOL), f32)
        M1p[8:8 + H, 1:1 + W] = m1[b]
        m1d = np.empty((128, 70, NCOL), BF)
        for rp in range(2):
            v = M1p[128 * half + 2 + rp:128 * half + 2 + rp + 140:2, :]
            m1d[64 * rp:64 * rp + 64] = np.broadcast_to(
                v.astype(BF)[None], (64, 70, NCOL))
        M4p = np.zeros((520, P4), f32)
        M4p[0:513, 0:513] = m4[b]
        # h4 is stored column-split: E block (even cols 0..512 -> 0:257),
        # O block (odd cols 1..511 -> 258:514); pad cols stay 0 so the
        # mask multiply also zeroes any never-written h4 garbage
        m4d = np.zeros((128, 129, P4), BF)
        for rp in range(2):
            v = M4p[256 * half + rp:256 * half + rp + 258:2, :]
            blk = np.zeros((129, P4), f32)
            blk[:, 0:257] = v[:, 0:514:2]
            blk[:, 258:514] = v[:, 1:513:2]
            m4d[64 * rp:64 * rp + 64] = np.broadcast_to(
                blk.astype(BF)[None], (64, 129, P4))
        in_maps.append(dict(
            xd=np.ascontiguousarray(xd),
            m1d=np.ascontiguousarray(m1d),
            m4d=np.ascontiguousarray(m4d),
            w1d=w1d, w2c=w2c, w3c=w3c, wtc=wtc, w5c=w5c, biases=biases,
        ))
    return in_maps


def _build_program():
    nc = bacc.Bacc("TRN2", target_bir_lowering=False, debug=False,
                   enable_asserts=True, num_devices=8)

    xd_d = nc.dram_tensor("xd", [128, NSLOT, NCOL], BF16,
                          kind="ExternalInput").ap()
    m1_d = nc.dram_tensor("m1d", [128, 70, NCOL], BF16,
                          kind="ExternalInput").ap()
    m4_d = nc.dram_tensor("m4d", [128, 129, P4], BF16,
                          kind="ExternalInput").ap()
    w1_d = nc.dram_tensor("w1d", [128, 3, 128], BF16,
                          kind="ExternalInput").ap()
    w2_d = nc.dram_tensor("w2c", [128, 6 if USE_STAG2 else 9, 128], BF16,
                          kind="ExternalInput").ap()
    w3_d = nc.dram_tensor("w3c", [128, 6 if USE_STAG3 else 9, 128], BF16,
                          kind="ExternalInput").ap()
    wt_d = nc.dram_tensor("wtc", [128, 3, 128], BF16,
                          kind="ExternalInput").ap()
    w5_d = nc.dram_tensor("w5c", [128, 6, 128], BF16,
                          kind="ExternalInput").ap()
    bias_d = nc.dram_tensor("biases", [128, 5], F32,
                            kind="ExternalInput").ap()
    out_d = nc.dram_tensor("out", [128, 128, 511], BF16,
                           kind="ExternalOutput").ap()

    with tile.TileContext(nc) as tc, ExitStack() as ctx:
        wp = ctx.enter_context(tc.tile_pool(name="wp", bufs=1))
        xp = ctx.enter_context(tc.tile_pool(name="xp", bufs=2))
        mp = ctx.enter_context(tc.tile_pool(name="mp", bufs=2))
        hp = ctx.enter_context(tc.tile_pool(name="hp", bufs=1))
        pp = ctx.enter_context(tc.tile_pool(name="pp", bufs=4, space="PSUM"))
        op = ctx.enter_context(tc.tile_pool(name="op", bufs=6))

        h1 = wp.tile([128, S1, NCOL], BF16, name="h1buf")
        h2 = wp.tile([128, S2, NCOL], BF16, name="h2buf")
        g1 = wp.tile([128, S2 + 1, NCOL], BF16, name="g1buf")
        g2 = wp.tile([128, S3 + 1, NCOL], BF16, name="g2buf")
        w1t = wp.tile([128, 3, 128], BF16, name="w1t")
        w2t = wp.tile([128, 6 if USE_STAG2 else 9, 128], BF16, name="w2t")
        w3t = wp.tile([128, 6 if USE_STAG3 else 9, 128], BF16, name="w3t")
        wtt = wp.tile([128, 3, 128], BF16, name="wtt")
        w5t = wp.tile([128, 6, 128], BF16, name="w5t")
        bt = wp.tile([128, 5], F32, name="bt")
        nc.sync.dma_start(w1t[:], w1_d[:])
        nc.sync.dma_start(bt[:], bias_d[:])
        nc.scalar.dma_start(w2t[:], w2_d[:])
        nc.scalar.dma_start(w3t[:], w3_d[:])
        nc.scalar.dma_start(wtt[:], wt_d[:])
        nc.scalar.dma_start(w5t[:], w5_d[:])

        def conv1_layer(x2, s_lo, m_ch, h_out):
            nc.gpsimd.memset(h_out[:, :, 0:NCOL:NCOL - 1], 0)
            for t0 in range(s_lo, S1, 2):
                nt = min(2, S1 - t0)
                pc = pp.tile([128, 2, 256], F32, name="pc", tag="ps")
                ps = pc[:, 0:nt, :]
                for d in range(3):
                    nc.tensor.matmul(
                        ps, w1t[:, d, :],
                        x2[:, t0:t0 + nt, d:d + 256],
                        start=(d == 0), stop=(d == 2))
                dst = h_out[:, t0:t0 + nt, 1:257]
                nc.scalar.activation(dst, ps, RELU, bias=bt[:, 0:1])
                nc.vector.tensor_mul(dst, dst, m_ch[:, t0:t0 + nt, 1:257])

        def stagger(g, h, s_lo, ns):
            # g[0:64, v] = h[0:64, v+1]; g[64:128, v] = h[64:128, v-1]
            nc.vector.tensor_copy(g[0:64, s_lo + 1:ns + 1, :],
                                  h[0:64, s_lo + 2:ns + 2, :])
            nc.vector.tensor_copy(g[64:128, s_lo + 1:ns + 1, :],
                                  h[64:128, s_lo:ns, :])

        def conv_layer(inp, g, wt_, s_lo, nslots, bias_ap, m_ch, moff, h_out,
                       use_stagger=True):
            nc.gpsimd.memset(h_out[:, :, 0:NCOL:NCOL - 1], 0)
            if use_stagger:
                stagger(g, inp, s_lo, nslots)
            for t0 in range(s_lo, nslots, 2):
                nt = min(2, nslots - t0)
                pc = pp.tile([128, 2, 256], F32, name="pc", tag="ps")
                ps = pc[:, 0:nt, :]
                if use_stagger:
                    for d in range(3):
                        nc.tensor.matmul(
                            ps, wt_[:, d, :],
                            inp[:, t0 + 1:t0 + 1 + nt, d:d + 256],
                            start=(d == 0), stop=False)
                    for d in range(3):
                        nc.tensor.matmul(
                            ps, wt_[:, 3 + d, :],
                            g[:, t0 + 1:t0 + 1 + nt, d:d + 256],
                            start=False, stop=(d == 2))
                else:
                    for k, (dlt, d) in enumerate(DELTAS9):
                        nc.tensor.matmul(
                            ps, wt_[:, k, :],
                            inp[:, t0 + 1 + dlt:t0 + 1 + dlt + nt, d:d + 256],
                            start=(k == 0), stop=(k == 8))
                dst = h_out[:, t0:t0 + nt, 1:257]
                nc.scalar.activation(dst, ps, RELU, bias=bias_ap)
                nc.vector.tensor_mul(
                    dst, dst, m_ch[:, t0 + moff:t0 + moff + nt, 1:257])

        def load_chunk(c):
            # xd slots are global out-slots 16c+t0; load fresh ones into
            # matching local positions so conv1 indexes t0 directly
            x_ch = xp.tile([128, S1, NCOL], BF16, name="x_ch", tag="x",
                           bufs=1)
            g0 = SADV * c
            ov1 = S1 - SADV
            if c == 0:
                nc.sync.dma_start(x_ch[:, 0:ov1, :], xd_d[:, 0:ov1, :])
            mid = ov1 + 8
            nc.sync.dma_start(x_ch[:, ov1:mid, :],
                              xd_d[:, g0 + ov1:g0 + mid, :])
            nc.sync.dma_start(x_ch[:, mid:S1, :], xd_d[:, g0 + mid:g0 + S1, :])
            m1_ch = mp.tile([128, S1, NCOL], BF16, name="m1_ch", tag="m1")
            nc.gpsimd.dma_start(m1_ch[:, 0:8, :],
                                m1_d[:, SADV * c:SADV * c + 8, :])
            nc.gpsimd.dma_start(m1_ch[:, 8:S1, :],
                                m1_d[:, SADV * c + 8:SADV * c + S1, :])
            return x_ch, m1_ch

        def load_m4(c):
            # deliberately emitted after conv1 so its bulk transfer queues
            # behind conv1's activations and never starves the startup loads
            m4_ch = mp.tile([128, NUP, P4], BF16, name="m4_ch", tag="m4",
                            bufs=1)
            nc.scalar.dma_start(m4_ch[:], m4_d[:, (CH // 2) * c:
                                               (CH // 2) * c + NUP, :])
            return m4_ch

        def convT_batch(h3, h3o, h4, l0, nb):
            T, ii = (h3o, l0 // 2) if l0 % 2 == 0 else (h3, (l0 + 1) // 2)
            pe = pp.tile([128, 2, 512], F32, name="pe", tag="ps")
            for q in range(nb):
                nc.tensor.matmul(pe[:, q, 0:257], wtt[:, 0, :],
                                 T[:, ii + q, 1:258],
                                 start=True, stop=False)
                nc.tensor.matmul(pe[:, q, 0:257], wtt[:, 1, :],
                                 T[:, ii + q, 0:257],
                                 start=False, stop=True)
            de = h4[:, l0:l0 + 2 * nb - 1:2, 0:257]
            nc.vector.tensor_scalar(de, pe[:, 0:nb, 0:257], bt[:, 3:4], 0.0,
                                    ADD, MAX)
            po = pp.tile([128, 2, 256], F32, name="po", tag="ps")
            nc.tensor.matmul(po[:, 0:nb, :], wtt[:, 2, :],
                             T[:, ii:ii + nb, 1:257],
                             start=True, stop=True)
            do = h4[:, l0:l0 + 2 * nb - 1:2, 258:514]
            nc.gpsimd.tensor_scalar(do, po[:, 0:nb, :], bt[:, 3:4], 0.0,
                                    ADD, MAX)

        def mask4(h4, m4_ch, p0, p1):
            nc.gpsimd.tensor_mul(h4[:, p0:p1, :], h4[:, p0:p1, :],
                                 m4_ch[:, p0:p1, :])

        def emit_convT(h3, h3o, h4, m4_ch):
            for l0, nb in TBATCH:
                convT_batch(h3, h3o, h4, l0, nb)
            for p0 in range(0, NUP, (NUP + 3) // 4):
                p1 = min(NUP, p0 + (NUP + 3) // 4)
                mask4(h4, m4_ch, p0, p1)

        def h4mv(h4, l, d):
            # conv5 moving operand over the E|O-split h4: psum col 2u+j
            # reads (E[u],O[u]) for d=0, (O[u],E[u+1]) for d=1,
            # (E[u+1],O[u+1]) for d=2
            base = h4[:, l, 0:2]
            pdim = list(base.ap[0])
            off = base.offset
            if d == 0:
                return AP(base.tensor, off, [pdim, [1, 256], [258, 2]])
            if d == 1:
                return AP(base.tensor, off + 258, [pdim, [1, 256], [-257, 2]])
            return AP(base.tensor, off + 1, [pdim, [1, 256], [258, 2]])

        def conv5_one(h4, c, r0):
            out_sb = op.tile([128, 2, 511], BF16, name="out_sb", tag="o")
            p5 = pp.tile([128, 2, 512], F32, name="p5", tag="ps")
            for q in range(2):
                for k, (sg, d) in enumerate(SIGD):
                    nc.tensor.matmul(p5[:, q, :], w5t[:, k, :],
                                     h4mv(h4, r0 + q + sg, d),
                                     start=(k == 0), stop=(k == 5))
            r = (r0 // 2) % 4
            if r in (0, 2):
                nc.scalar.activation(out_sb[:], p5[:, :, 0:511], RELU,
                                     bias=bt[:, 4:5])
            elif r == 1:
                nc.vector.tensor_scalar(out_sb[:], p5[:, :, 0:511],
                                        bt[:, 4:5], 0.0, ADD, MAX)
            else:
                nc.gpsimd.tensor_scalar(out_sb[:], p5[:, :, 0:511],
                                        bt[:, 4:5], 0.0, ADD, MAX)
            eng = nc.gpsimd if (r0 // 2) % 2 == 0 else nc.sync
            eng.dma_start(
                out_d[:, (CH // 2) * c + r0:(CH // 2) * c + r0 + 2, :],
                out_sb[:])

        def emit_conv5(h4, c, r0s):
            for r0 in r0s:
                conv5_one(h4, c, r0)

        def emit_tail(h3, h3o, h4, m4_ch, c):
            # final chunk: interleave convT batches, mask muls, and conv5
            # row-groups so the tensor engine never drains at the tail
            for k in range(8):
                convT_batch(h3, h3o, h4, 4 * k, 2)
                convT_batch(h3, h3o, h4, 4 * k + 1, 2)
                mask4(h4, m4_ch, 4 * k, 4 * k + 4)
                if k > 0:
                    conv5_one(h4, c, 4 * k - 2)
                conv5_one(h4, c, 4 * k)
            convT_batch(h3, h3o, h4, 32, 1)
            mask4(h4, m4_ch, 32, 33)
            conv5_one(h4, c, 30)

        prev = None  # (h3, h3o, h4, m4_ch, c)
        h3prev = None
        nxt = load_chunk(0)
        for it in range(NCH + 1):
            if it < NCH:
                x_ch, m1_ch = nxt
                if it + 1 < NCH:
                    nxt = load_chunk(it + 1)
                h3 = hp.tile([128, S3, NCOL], BF16, name="h3", tag="h3",
                             bufs=2)
                h3o = hp.tile([128, S3 - 1, NCOL], BF16, name="h3o",
                              tag="h3o", bufs=2)
                # carry the exact boundary slots from the previous chunk
                # instead of recomputing them (h1/h2 are persistent tiles;
                # in-place copies between disjoint slot ranges)
                ov1, ov2, ov3 = S1 - SADV, S2 - SADV, S3 - SADV
                if it > 0:
                    nc.vector.tensor_copy(h1[:, 0:ov1, :],
                                          h1[:, SADV:S1, :])
                    nc.vector.tensor_copy(h2[:, 0:ov2, :],
                                          h2[:, SADV:S2, :])
                    nc.vector.tensor_copy(h3[:, 0:ov3, :],
                                          h3prev[:, SADV:S3, :])
                s1, s2, s3 = (ov1, ov2, ov3) if it > 0 else (0, 0, 0)
                conv1_layer(x_ch, s1, m1_ch, h1)
                m4_ch = load_m4(it)
            if prev is not None:
                h3p, h3op, h4p, m4p_, cp = prev
                if it == NCH:
                    emit_tail(h3p, h3op, h4p, m4p_, cp)
                else:
                    emit_convT(h3p, h3op, h4p, m4p_)
            if it < NCH:
                conv_layer(h1, g1, w2t, s2, S2, bt[:, 1:2], m1_ch, 1, h2,
                           use_stagger=USE_STAG2)
            if prev is not None and it < NCH:
                emit_conv5(h4p, cp, range(0, CH // 4, 2))
            if it < NCH:
                conv_layer(h2, g2, w3t, s3, S3, bt[:, 2:3], m1_ch, 2, h3,
                           use_stagger=USE_STAG3)
                nc.vector.tensor_copy(h3o[0:64, :, :],
                                      h3[64:128, 0:S3 - 1, :])
                nc.vector.tensor_copy(h3o[64:128, :, :], h3[0:64, 1:S3, :])
            if prev is not None and it < NCH:
                emit_conv5(h4p, cp, range(CH // 4, CH // 2, 2))
            if it < NCH:
                h4 = hp.tile([128, NUP, P4], BF16, name="h4", tag="h4")
                prev = (h3, h3o, h4, m4_ch, it)
                h3prev = h3
            else:
                prev = None

    nc.compile()
    return nc


def kernel(**inputs):
    features = np.asarray(inputs["features"], np.float32)
    coors = np.asarray(inputs["coors"], np.int32)
    args = [np.asarray(inputs[k], np.float32) for k in
            ("w1", "b1", "w2", "b2", "w3", "b3", "wt", "bt", "w5", "b5")]
    in_maps = _host_prep(features, coors, *args)
    if "nc" not in _CACHE:
        _CACHE["nc"] = _build_program()
    res = run_bass_kernel_spmd(_CACHE["nc"], in_maps,
                               core_ids=list(range(8)), trace=False)
    full = np.zeros((B, 511, 511, 64), np.float32)
    for core in range(8):
        b, half = core // 2, core % 2
        o = np.asarray(res.results[core]["out"]).astype(np.float32)
        rows = o.reshape(2, 64, 128, 511).transpose(2, 0, 3, 1) \
            .reshape(256, 511, 64)
        nrow = 256 if half == 0 else 255
        full[b, 256 * half:256 * half + nrow] = rows[:nrow]
    return full
